# revision 20
# baseline (speedup 1.0000x reference)
"""Trainium2 Bass kernel for nn_MemoryNetwork (scatter_memory).

Computation (reference, per batch row b):
    f = feature / ||feature||                       [B, 768]
    topic = f @ W_topic.T ; dom = f @ W_domain.T    [B, 256]
    att   = softmax_m(TAU * topic . memory[d,m])    [B, 9, 10]
    sep   = sum_m att * memory[d,m]                 [B, 9, 256]
    out   = softmax_d(TAU * sep . dom)              [B, 1, 9]

Reformulation: the memory banks are tiny, so fold them into the projection
weights on the host:
    P = mem_flat @ W_topic ; Q = mem_flat @ W_domain ; R = [P; Q]  [180, 768]
Per row only one [768 x 180] product is needed:
    raw    = feature @ R.T                   (rawS | rawT)
    r      = TAU / ||feature||               (= exp(-0.5 ln(norm2) + ln TAU))
    ex     = exp(rawS * r - SHIFT)           (softmax_m numerator, const shift
                                              instead of max-subtraction; safe:
                                              logits are in [-130, 110])
    sums_d = sum_m ex ; wsum_d = sum_m ex * rawT
    datt   = (wsum / sums) * r               (= TAU * domain_att)
    out    = softmax_d(datt)                 (const shift again)

Sharding: data-parallel over B across 8 cores (4096 rows each); R.T
replicated. Features are sent in transposed layout [768, 4096] per core so
the PE needs no on-chip transposes (matmul contracts over partitions = IN).
Matmuls run in exact fp32. Row norms are computed on-device (square split
across ACT/DVE + PE ones-matmul reduction).
"""

import sys

sys.path.insert(0, "/opt/trn_rl_repo")

import numpy as np

B, IN, E, D, M = 32768, 768, 256, 9, 10
NCORES = 8
BC = B // NCORES  # rows per core
P = 128           # partition tile
NT = BC // P      # batch tiles per core (32)
G = 8             # tiles per softmax group
NG = NT // G
DM = 2 * D * M    # 180
USE_F32R = False  # f32r: 4x faster PE matmul but ~12-bit input rounding
NPAD = 256 if USE_F32R else DM
KC = IN // P      # contraction chunks (6)
TAU = 32.0
SHIFT = 50.0
LN_TAU = float(np.log(TAU))

_CACHE: dict = {}


def _build_nc(repeat=1):
    from contextlib import ExitStack

    import concourse.bacc as bacc
    import concourse.tile as tile
    from concourse import mybir

    F32 = mybir.dt.float32
    F32R = mybir.dt.float32r
    AF = mybir.ActivationFunctionType
    FMM = F32R if USE_F32R else F32

    nc = bacc.Bacc(trn_type="TRN2")
    ft = nc.dram_tensor("ft", [IN, BC], F32, kind="ExternalInput")
    rt = nc.dram_tensor("rt", [IN, NPAD], FMM, kind="ExternalInput")
    out = nc.dram_tensor("out", [BC, D], F32, kind="ExternalOutput")

    with tile.TileContext(nc) as tc, ExitStack() as ctx:
        const = ctx.enter_context(tc.tile_pool(name="const", bufs=1))
        fpool = ctx.enter_context(tc.tile_pool(name="fts", bufs=G + 2))
        sqpool = ctx.enter_context(tc.tile_pool(name="sq", bufs=3))
        gpool = ctx.enter_context(tc.tile_pool(name="grp", bufs=2))
        spool = ctx.enter_context(tc.tile_pool(name="small", bufs=2))
        n2_ps = ctx.enter_context(tc.tile_pool(name="n2ps", bufs=2, space="PSUM"))
        raw_ps = ctx.enter_context(tc.tile_pool(name="rawps", bufs=4, space="PSUM"))

        # Constants
        rt_sb = const.tile([P, KC, NPAD], FMM)
        nc.sync.dma_start(rt_sb[:], rt[:, :].rearrange("(k p) j -> p k j", p=P))
        ones = const.tile([P, 1], F32)
        nc.gpsimd.memset(ones[:], 1.0)
        bias_lntau = const.tile([P, 1], F32)
        nc.gpsimd.memset(bias_lntau[:], LN_TAU)
        bias_shift = const.tile([P, 1], F32)
        nc.gpsimd.memset(bias_shift[:], -SHIFT)
        out_sb = const.tile([P, NT, D], F32)

        # ft viewed as [p, k, b_global]
        ft_v = ft[:, :].rearrange("(k p) b -> p k b", p=P)

        for g in range(NG * repeat):
            g = g % NG
            norm2 = gpool.tile([P, G], F32, tag="norm2")
            lng = gpool.tile([P, G], F32, tag="lng")
            r_g = gpool.tile([P, G], F32, tag="rg")
            ex_g = gpool.tile([P, G, D * M], F32, tag="exg")
            t_g = gpool.tile([P, G, D * M], F32, tag="tg")

            # Pass 1: loads + row norms (dependencies of the grouped r calc)
            f_tiles = []
            for s in range(G):
                t = g * G + s
                ft_sb = fpool.tile([P, KC, P], F32, tag="ft")
                dma_eng = nc.sync if t % 2 == 0 else nc.scalar
                dma_eng.dma_start(ft_sb[:], ft_v[:, :, t * P : (t + 1) * P])
                f_tiles.append(ft_sb)

                # norm2: square (ACT half / DVE half), then PE ones-matmuls
                sq = sqpool.tile([P, KC, P], F32, tag="sq")
                nc.scalar.activation(sq[:, 0:3, :], ft_sb[:, 0:3, :], AF.Square)
                nc.vector.tensor_mul(sq[:, 3:6, :], ft_sb[:, 3:6, :], ft_sb[:, 3:6, :])
                n2p = n2_ps.tile([P, 1], F32, tag="n2")
                for k in range(KC):
                    nc.tensor.matmul(
                        n2p[:],
                        sq[:, k, :],
                        ones[:],
                        start=(k == 0),
                        stop=(k == KC - 1),
                    )
                nc.scalar.copy(norm2[:, s : s + 1], n2p[:])

            # r = TAU / sqrt(norm2) = exp(-0.5 * ln(norm2) + ln(TAU))
            nc.scalar.activation(lng[:], norm2[:], AF.Ln)
            nc.scalar.activation(
                r_g[:], lng[:], AF.Exp, bias=bias_lntau[:], scale=-0.5
            )

            # Pass 2: projection matmuls + softmax_m numerators
            for s in range(G):
                ft_sb = f_tiles[s]
                raw = raw_ps.tile([P, NPAD], F32, tag="raw")
                for k in range(KC):
                    nc.tensor.matmul(
                        raw[:],
                        ft_sb[:, k, :] if not USE_F32R else ft_sb[:, k, :].bitcast(F32R),
                        rt_sb[:, k, :],
                        start=(k == 0),
                        stop=(k == KC - 1),
                    )
                nc.scalar.activation(
                    ex_g[:, s, :],
                    raw[:, 0 : D * M],
                    AF.Exp,
                    bias=bias_shift[:],
                    scale=r_g[:, s : s + 1],
                )
                nc.scalar.copy(t_g[:, s, :], raw[:, D * M : DM])

            # Grouped softmax tail
            sums = spool.tile([P, G, D], F32, tag="sums")
            nc.vector.reduce_sum(
                sums[:],
                ex_g[:].rearrange("p s (d m) -> p s d m", d=D, m=M),
                axis=mybir.AxisListType.X,
            )
            prod = spool.tile([P, G, D * M], F32, tag="prod")
            nc.vector.tensor_mul(prod[:], ex_g[:], t_g[:])
            wsum = spool.tile([P, G, D], F32, tag="wsum")
            nc.vector.reduce_sum(
                wsum[:],
                prod[:].rearrange("p s (d m) -> p s d m", d=D, m=M),
                axis=mybir.AxisListType.X,
            )
            rsums = spool.tile([P, G, D], F32, tag="rsums")
            nc.vector.reciprocal(rsums[:], sums[:])
            datt0 = spool.tile([P, G, D], F32, tag="datt0")
            nc.vector.tensor_mul(datt0[:], wsum[:], rsums[:])
            datt = spool.tile([P, G, D], F32, tag="datt")
            nc.vector.tensor_mul(
                datt[:], datt0[:], r_g[:, :, None].broadcast_to([P, G, D])
            )
            ex2 = spool.tile([P, G, D], F32, tag="ex2")
            nc.scalar.activation(ex2[:], datt[:], AF.Exp, bias=bias_shift[:])
            sumd = spool.tile([P, G], F32, tag="sumd")
            nc.vector.reduce_sum(sumd[:], ex2[:], axis=mybir.AxisListType.X)
            rd = spool.tile([P, G], F32, tag="rd")
            nc.vector.reciprocal(rd[:], sumd[:])
            nc.vector.tensor_mul(
                out_sb[:, g * G : (g + 1) * G, :],
                ex2[:],
                rd[:, :, None].broadcast_to([P, G, D]),
            )

            if g == NG - 1:
                nc.sync.dma_start(
                    out[:, :].rearrange("(t p) d -> p t d", p=P), out_sb[:]
                )

    nc.finalize()
    return nc


def _get_nc():
    if "nc" not in _CACHE:
        _CACHE["nc"] = _build_nc()
    return _CACHE["nc"]


def _host_rt(W_topic, W_domain, memory):
    mem_flat = memory.reshape(D * M, E).astype(np.float64)
    Pm = mem_flat @ W_topic.astype(np.float64)
    Qm = mem_flat @ W_domain.astype(np.float64)
    R = np.concatenate([Pm, Qm], axis=0).astype(np.float32)  # [180, 768]
    RT = np.zeros((IN, NPAD), dtype=np.float32)
    RT[:, :DM] = R.T
    return RT


def kernel(feature, category, W_topic, W_domain, memory):
    from concourse.bass_utils import run_bass_kernel_spmd

    feature = np.asarray(feature, dtype=np.float32)
    RT = _host_rt(np.asarray(W_topic), np.asarray(W_domain), np.asarray(memory))

    nc = _get_nc()
    in_maps = [
        {
            "ft": np.ascontiguousarray(feature[c * BC : (c + 1) * BC].T),
            "rt": RT,
        }
        for c in range(NCORES)
    ]
    res = run_bass_kernel_spmd(nc, in_maps, core_ids=list(range(NCORES)))
    outs = [res.results[c]["out"] for c in range(NCORES)]
    full = np.concatenate(outs, axis=0)  # [B, 9]
    return full[:, None, :].astype(np.float32)
